# revision 28
# baseline (speedup 1.0000x reference)
"""Trainium2 Bass kernel for CrossAttention (LayerNorm + self-attention + 1x1 conv + residual).

Sharding: data-parallel over batch - B=8, one batch element per NeuronCore.

Per-core design:
 - ScalarE exp stream is the roofline (H*L^2 = 16.8M exps @ 1 elem/cycle/lane
   @1.2GHz ~= 110us + per-instruction overhead); everything else is pipelined
   underneath it, so the optimization targets are the head (time before the
   first exp) and the tail.
 - weights are folded on the host: (g*W)^T fp16, LN-mean/bias augmentation rows,
   Wo^T fp16 - the device does no weight transposes or scaling at all.
 - LayerNorm + QKV projection run in 4 column-chunks of 512 software-pipelined
   INTO the attention loop: chunk 0 gates the first exp (~15us head), chunks
   1-3 are emitted between the first exp slots of d=0 and execute underneath
   the exp stream on PE/DVE/Pool.
 - all matmuls fp16 (1 cycle/row on PE), fp32 PSUM accumulation; softmax
   without max-subtraction (logits are O(1)).
 - scores for 2 heads x 512 queries per [128,1024] PSUM tile, 3-deep rotation;
   Z/AV matmuls run one pipeline position behind the exps.
 - a short chained-matmul warmup burst runs during the x-DMA wait so the PE
   HAM clock-gate opens (1.2 -> 2.4 GHz) before the attention stream starts.
 - single ACT table load: the ln/exp loads are steered to the combined
   natural_log_exp_and_others set (see _patch_act_tables).
"""
import numpy as np

B, C, L = 8, 256, 2048
H, DH = 4, 32
HID = H * DH           # 128
EPS = 1e-5
SCALE = DH ** -0.5
P = 128                # partitions
DQ = 512               # query tile (free dim of score matmuls)
ND = L // DQ           # 4 d-tiles
NE = L // P            # 16 key tiles of 128
CW = 512               # LN/proj column chunk width
NC = L // CW           # 4 chunks

_cached = None


def _patch_act_tables():
    """Steer the greedy ACT-table-load inserter to the combined ln+exp set.

    The inserter picks the first act_func_set containing the needed function;
    'exp' hits exp_and_others and 'ln' hits natural_log, so an interleaved
    ln/exp sequence reloads tables on every switch (1.28us each). Dropping
    those functions from the other sets (indices preserved) makes both
    resolve to natural_log_exp_and_others -> one load for the whole kernel.
    Set ids are positional, so only contents may be edited.
    """
    import concourse.bacc as bacc

    if getattr(bacc, '_act_tables_patched', False):
        return
    orig = bacc.get_activation_tables

    def patched(arch):
        tables = {k: set(v) for k, v in orig(arch).items()}
        if 'natural_log_exp_and_others' in tables:
            combined = tables['natural_log_exp_and_others']
            for name, fns in tables.items():
                if name != 'natural_log_exp_and_others':
                    fns -= {f for f in fns if f in combined
                            and str(f).lower().endswith(('exp', 'ln', 'square'))}
        return tables

    bacc.get_activation_tables = patched
    bacc._act_tables_patched = True


def _build():
    import concourse.bass as bass
    import concourse.bacc as bacc
    import concourse.tile as tile
    from concourse import mybir

    _patch_act_tables()

    f32 = mybir.dt.float32
    f16 = mybir.dt.float16
    AF = mybir.ActivationFunctionType
    OP = mybir.AluOpType

    nc = bacc.Bacc('TRN2', target_bir_lowering=False, debug=False, num_devices=B)

    xd = nc.dram_tensor('x', [C, L], f32, kind='ExternalInput').ap()
    wqkvd = nc.dram_tensor('wqkvT', [C, 3 * HID], f16, kind='ExternalInput').ap()
    aug3d = nc.dram_tensor('aug3', [2, 3 * HID], f16, kind='ExternalInput').ap()
    wod = nc.dram_tensor('woT', [HID, C], f16, kind='ExternalInput').ap()
    bod = nc.dram_tensor('bo', [C, 1], f32, kind='ExternalInput').ap()
    yd = nc.dram_tensor('y', [C, L], f32, kind='ExternalOutput').ap()

    with tile.TileContext(nc) as tc:
        with (
            tc.tile_pool(name='const', bufs=1) as const,
            tc.tile_pool(name='big', bufs=1) as big,
            tc.tile_pool(name='sc', bufs=2) as sc,
            tc.tile_pool(name='apool', bufs=6) as apool,
            tc.tile_pool(name='tpool', bufs=2) as tpool,
            tc.tile_pool(name='psA', bufs=3, space='PSUM') as psA,
            tc.tile_pool(name='psB', bufs=2, space='PSUM') as psB,
        ):
            # ---- persistent tiles ----
            x0 = big.tile([P, L], f32, tag='x0')
            x1 = big.tile([P, L], f32, tag='x1')
            xs0 = big.tile([P, L], f16, tag='xs0')
            xs1 = big.tile([P, L], f16, tag='xs1')
            qt = big.tile([HID, L], f16, tag='qt')
            kt = big.tile([HID, L], f16, tag='kt')
            vsb = big.tile([P, NE, HID], f16, tag='vsb')
            aug2 = big.tile([2, L], f16, tag='aug2')

            wqkv0 = const.tile([P, 3 * HID], f16, tag='wqkv0')
            wqkv1 = const.tile([P, 3 * HID], f16, tag='wqkv1')
            aug3sb = const.tile([2, 3 * HID], f16, tag='aug3sb')
            woT = const.tile([HID, C], f16, tag='woT')
            boc = [const.tile([P, 1], f32, tag=f'bo{c}', name=f'bo{c}') for c in range(2)]
            ones16 = const.tile([P, P], f16, tag='ones16')
            epst = const.tile([P, 1], f32, tag='epst')
            warm_src = const.tile([P, 512], f16, tag='wsrc')

            WIX = {'q': 0, 'k': 1, 'v': 2}

            def wsl(name, c):
                i = WIX[name]
                t = wqkv0 if c == 0 else wqkv1
                return t[:, i * HID:(i + 1) * HID]

            def asl(name):
                i = WIX[name]
                return aug3sb[:, i * HID:(i + 1) * HID]

            # ---- queue heads: x chunk0 first on SyncE; weights on Tensor/Vector
            # queues; ScalarE queue stays empty so the ACT table load fires
            # immediately and the LN chain is never behind DMA issues ----
            nc.sync.dma_start(out=x0[:, 0:CW], in_=xd[0:P, 0:CW])
            nc.sync.dma_start(out=x1[:, 0:CW], in_=xd[P:C, 0:CW])
            nc.sync.dma_start(out=x0[:, CW:L], in_=xd[0:P, CW:L])
            nc.sync.dma_start(out=x1[:, CW:L], in_=xd[P:C, CW:L])
            nc.scalar.dma_start(out=wqkv0, in_=wqkvd[0:P, :])
            nc.scalar.dma_start(out=wqkv1, in_=wqkvd[P:C, :])
            nc.vector.memset(epst, EPS)
            nc.scalar.dma_start(out=woT, in_=wod)
            nc.scalar.dma_start(out=aug3sb, in_=aug3d)
            nc.scalar.dma_start(out=boc[0], in_=bod[0:P, :])
            nc.scalar.dma_start(out=boc[1], in_=bod[P:C, :])
            nc.gpsimd.memset(warm_src, 0.5)
            nc.gpsimd.memset(ones16, 1.0)
            nc.gpsimd.memset(aug2, 1.0)  # row 1 stays 1.0; row 0 overwritten per chunk

            # ---- PE warmup: chained dead matmuls during the x-DMA wait; the
            # HAM clock gate needs >=3.4us of sustained busy to open
            # (1.2 -> 2.4 GHz), and a ~3.4us idle re-throttles it, so the
            # initial burst is >4us and short filler bursts are interleaved
            # with the sparse chunk-0 PE work to bridge until the attention
            # stream (which then keeps PE dense) ----
            # The warm matmuls write garbage into d=0's zp accumulator
            # (pre-allocated here): zp0 is read by the softmax reciprocal so
            # the chain is output-rooted (a dedicated scratch tile gets
            # dead-code-eliminated, silently dropping the warmup), and the
            # first real ZAV matmul has start=True, which clears the bank.
            zp0 = psB.tile([P, DQ], f32, tag='acc', name='zp0')
            op0 = psB.tile([P, DQ], f32, tag='acc', name='op0')
            NWARM = 10
            for i in range(NWARM):
                nc.tensor.matmul(zp0, lhsT=ones16, rhs=warm_src,
                                 start=(i == 0), stop=False)

            def emit_warm_filler(n, close=False):
                for i in range(n):
                    nc.tensor.matmul(zp0, lhsT=ones16, rhs=warm_src,
                                     start=False, stop=(close and i == n - 1))

            # ---- per-chunk LN + projections ----
            def emit_chunk_A(c, act_sq=False):
                """stats -> mean, rstd for columns [c*CW, (c+1)*CW).

                act_sq: compute the squares on the (idle) ScalarE instead of
                Pool - used for chunk 0 only, where the squares sit on the
                critical path to the first exp and ACT has nothing to do yet;
                Square lives in the combined ln/exp table set (see
                _patch_act_tables) so it costs no extra table load.
                """
                sl = slice(c * CW, (c + 1) * CW)
                xb0 = sc.tile([P, CW], f16, tag='xb', name=f'xb0_{c}')
                xb1 = sc.tile([P, CW], f16, tag='xb2', name=f'xb1_{c}')
                xq0 = sc.tile([P, CW], f16, tag='xq', name=f'xq0_{c}')
                xq1 = sc.tile([P, CW], f16, tag='xq2', name=f'xq1_{c}')
                if act_sq:
                    # chunk 0: casts on the (empty) DVE queue, squares on the
                    # idle ScalarE
                    nc.vector.tensor_copy(xb0, x0[:, sl])
                    nc.vector.tensor_copy(xb1, x1[:, sl])
                    nc.scalar.activation(xq0, x0[:, sl], AF.Square)
                    nc.scalar.activation(xq1, x1[:, sl], AF.Square)
                else:
                    # chunks 1-3: xb casts go to Pool. They are the only
                    # chunk ops whose inputs are ready early (just x), so the
                    # Tile scheduler otherwise promotes them ahead of chunk
                    # 0's veps in the in-order DVE queue, delaying the first
                    # Ln (and first exp) by ~4us. Everything else left on DVE
                    # waits on late PE/ACT results and cannot be promoted.
                    nc.gpsimd.tensor_scalar_mul(xb0, x0[:, sl], 1.0)
                    nc.gpsimd.tensor_scalar_mul(xb1, x1[:, sl], 1.0)
                    nc.gpsimd.tensor_mul(xq0, x0[:, sl], x0[:, sl])
                    nc.gpsimd.tensor_mul(xq1, x1[:, sl], x1[:, sl])
                st = psA.tile([P, 1024], f32, tag='psA', name=f'st{c}')
                nc.tensor.matmul(st[:, 0:CW], lhsT=ones16, rhs=xb0, start=True, stop=False)
                nc.tensor.matmul(st[:, 0:CW], lhsT=ones16, rhs=xb1, start=False, stop=True)
                nc.tensor.matmul(st[:, CW:], lhsT=ones16, rhs=xq0, start=True, stop=False)
                nc.tensor.matmul(st[:, CW:], lhsT=ones16, rhs=xq1, start=False, stop=True)
                mean = sc.tile([P, CW], f32, tag='mean', name=f'mean{c}')
                nc.vector.tensor_scalar_mul(mean, st[:, 0:CW], 1.0 / C)
                # msq on DVE: shorter serial chain than Pool (690 vs 1250ns)
                # and frees Pool capacity for the chunk 1-3 xb casts above
                msq = sc.tile([P, CW], f32, tag='msq', name=f'msq{c}')
                nc.vector.tensor_mul(msq, mean, mean)
                veps = sc.tile([P, CW], f32, tag='veps', name=f'veps{c}')
                nc.vector.scalar_tensor_tensor(veps, in0=st[:, CW:], scalar=1.0 / C,
                                               in1=msq, op0=OP.mult, op1=OP.subtract)
                lv = sc.tile([P, CW], f32, tag='lv', name=f'lv{c}')
                nc.scalar.activation(lv, veps, AF.Ln, bias=epst)
                # rstd = exp(-0.5*ln(var+eps)); single combined ln/exp ACT table
                rstd = sc.tile([P, CW], f32, tag='rstd', name=f'rstd{c}')
                nc.scalar.activation(rstd, lv, AF.Exp, scale=-0.5)
                return mean, rstd

            def emit_proj(name, dst, c):
                sl = slice(c * CW, (c + 1) * CW)
                pt = psA.tile([P, CW], f32, tag='psA', name=f'pt_{name}{c}')
                nc.tensor.matmul(pt, lhsT=wsl(name, 0), rhs=xs0[:, sl], start=True, stop=False)
                nc.tensor.matmul(pt, lhsT=wsl(name, 1), rhs=xs1[:, sl], start=False, stop=False)
                nc.tensor.matmul(pt, lhsT=asl(name), rhs=aug2[:, sl], start=False, stop=True)
                nc.vector.tensor_copy(dst[:, sl], pt)

            def emit_chunk_B1(c, mean, rstd):
                """xs, aug row, q projection for the chunk."""
                sl = slice(c * CW, (c + 1) * CW)
                nc.vector.tensor_mul(xs0[:, sl], x0[:, sl], rstd)
                nc.vector.tensor_mul(xs1[:, sl], x1[:, sl], rstd)
                nc.gpsimd.tensor_mul(aug2[0:1, sl], mean[0:1, :], rstd[0:1, :])
                emit_proj('q', qt, c)

            def emit_chunk_B2(c):
                """k projection for the chunk."""
                emit_proj('k', kt, c)

            def emit_chunk_B(c, mean, rstd):
                emit_chunk_B1(c, mean, rstd)
                emit_chunk_B2(c)

            def emit_v_chunk(e):
                se = slice(e * P, (e + 1) * P)
                vp = psA.tile([P, HID], f32, tag='psA', name=f'vp{e}')
                nc.tensor.matmul(vp, lhsT=xs0[:, se], rhs=wsl('v', 0), start=True, stop=False)
                nc.tensor.matmul(vp, lhsT=xs1[:, se], rhs=wsl('v', 1), start=False, stop=False)
                nc.tensor.matmul(vp, lhsT=aug2[:, se], rhs=asl('v'), start=False, stop=True)
                nc.vector.tensor_copy(vsb[:, e, :], vp)

            # ---- attention: pipelined over (d, e); ACT exp stream is the roofline ----
            def emit_qk_exp(d, e):
                sd = slice(d * DQ, (d + 1) * DQ)
                se = slice(e * P, (e + 1) * P)
                ats = []
                for half in range(2):
                    sp = psA.tile([P, 1024], f32, tag='psA', name=f'sp{d}_{e}_{half}')
                    for hh in range(2):
                        h = half * 2 + hh
                        hp = slice(32 * h, 32 * h + 32)
                        nc.tensor.matmul(sp[:, hh * DQ:(hh + 1) * DQ], lhsT=kt[hp, se],
                                         rhs=qt[hp, sd], start=True, stop=True,
                                         tile_position=(32 * h, 0))
                    at = apool.tile([P, 1024], f16, tag='at', name=f'at{d}_{e}_{half}')
                    nc.scalar.activation(at, sp, AF.Exp, scale=SCALE)
                    ats.append(at)
                return ats

            def emit_zav(d, e, ats, zp, op_):
                for half in range(2):
                    at = ats[half]
                    for hh in range(2):
                        h = half * 2 + hh
                        hp = slice(32 * h, 32 * h + 32)
                        asl_ = slice(hh * DQ, (hh + 1) * DQ)
                        nc.tensor.matmul(zp[hp, :], lhsT=ones16[:, 0:32], rhs=at[:, asl_],
                                         start=(e == 0), stop=(e == NE - 1),
                                         tile_position=(0, 32 * h))
                        nc.tensor.matmul(op_[hp, :], lhsT=vsb[:, e, hp], rhs=at[:, asl_],
                                         start=(e == 0), stop=(e == NE - 1),
                                         tile_position=(0, 32 * h))

            def emit_norm_tail(d, zp, op_):
                # ~18-bit reciprocal, 5x faster than nc.vector.reciprocal --
                # plenty for softmax denominators (Z is O(100..3000), positive)
                rz = tpool.tile([P, DQ], f32, tag='rz', name=f'rz{d}')
                nc.vector.reciprocal_approx_fast(out=rz, in_=zp)
                onorm = tpool.tile([P, DQ], f16, tag='onorm', name=f'onorm{d}')
                nc.vector.tensor_mul(onorm, op_, rz)
                return onorm

            def emit_proj_tail(d, onorm):
                sd = slice(d * DQ, (d + 1) * DQ)
                yp = psA.tile([P, 1024], f32, tag='psA', name=f'yp{d}')
                for c in range(2):
                    nc.tensor.matmul(yp[:, c * DQ:(c + 1) * DQ], lhsT=woT[:, c * P:(c + 1) * P],
                                     rhs=onorm, start=True, stop=True)
                for c in range(2):
                    ysb = tpool.tile([P, DQ], f32, tag='ysb', name=f'ysb{d}_{c}')
                    xc = x0 if c == 0 else x1
                    nc.vector.scalar_tensor_tensor(ysb, in0=yp[:, c * DQ:(c + 1) * DQ],
                                                   scalar=boc[c], in1=xc[:, sd],
                                                   op0=OP.add, op1=OP.add)
                    nc.sync.dma_start(out=yd[c * P:(c + 1) * P, sd], in_=ysb)

            # ---- chunk 0 gates the head; chunks 1-3 pipeline into d=0.
            # Warm fillers bridge the PE-idle stretches of the chunk-0 chain
            # (stats -> LN -> xs -> proj) so the HAM gate, opened by the
            # initial burst, is still open when the attention stream starts. ----
            m0, r0 = emit_chunk_A(0, act_sq=True)
            emit_warm_filler(6)
            emit_chunk_B(0, m0, r0)
            emit_warm_filler(3)
            for e in range(4):
                emit_v_chunk(e)
            emit_warm_filler(3, close=True)

            pend = None          # (c, mean, rstd) between chunk A and B
            prev = None          # (d, e, ats) whose ZAV is emitted one position late
            zp = op_ = None      # PSUM accumulators of prev's d
            pending_norm = None  # (d, onorm) waiting for its out-projection
            for d in range(ND):
                for e in range(NE):
                    # LN chunk pipeline for chunks 1-3, spread over 3 slots of
                    # d=0 so a (possibly still cold) PE is never oversubscribed
                    # in any one slot: A at e=4c-3, B split q/k at e=4c-2/4c-1.
                    # Emitted before this slot's QK/exp so the chunk's Ln/rstd
                    # sit in the ACT queue ahead of the exps that depend on
                    # the chunk's kt (needed first at e=4c).
                    if d == 0 and e in (1, 5, 9):
                        c = (e + 3) // 4
                        pend = (c,) + emit_chunk_A(c)
                    elif d == 0 and e in (2, 6, 10):
                        emit_chunk_B1(pend[0], pend[1], pend[2])
                    elif d == 0 and e in (3, 7, 11):
                        emit_chunk_B2(pend[0])
                        pend = None
                    ats = emit_qk_exp(d, e)
                    if d == 0 and e >= 4:
                        emit_v_chunk(e)
                    if prev is not None:
                        emit_zav(prev[0], prev[1], prev[2], zp, op_)
                        if prev[1] == NE - 1:
                            pending_norm = (prev[0], emit_norm_tail(prev[0], zp, op_))
                    if e == 0:
                        if d == 0:
                            zp, op_ = zp0, op0  # pre-allocated (warm-chain target)
                        else:
                            # allocate this d's accumulators AFTER the previous
                            # d's reciprocal was emitted (correct WAR on psB)
                            zp = psB.tile([P, DQ], f32, tag='acc', name=f'zp{d}')
                            op_ = psB.tile([P, DQ], f32, tag='acc', name=f'op{d}')
                    elif e == 2 and pending_norm is not None:
                        emit_proj_tail(pending_norm[0], pending_norm[1])
                        pending_norm = None
                    prev = (d, e, ats)
            # drain
            emit_zav(prev[0], prev[1], prev[2], zp, op_)
            onorm_last = emit_norm_tail(ND - 1, zp, op_)
            emit_proj_tail(ND - 1, onorm_last)

    nc.compile()
    return nc


def _get_nc():
    global _cached
    if _cached is None:
        _cached = _build()
    return _cached


def _prep_in_maps(inputs):
    x = np.ascontiguousarray(np.asarray(inputs['x'], dtype=np.float32))
    g = np.asarray(inputs['g'], dtype=np.float32).reshape(C)
    b = np.asarray(inputs['b'], dtype=np.float32).reshape(C)
    Wq = np.asarray(inputs['Wq'], dtype=np.float32)
    Wk = np.asarray(inputs['Wk'], dtype=np.float32)
    Wv = np.asarray(inputs['Wv'], dtype=np.float32)
    Wo = np.asarray(inputs['Wo'], dtype=np.float32)
    bo = np.ascontiguousarray(np.asarray(inputs['bo'], dtype=np.float32).reshape(C, 1))

    # host-side weight folding: LN affine (g, b) into the QKV projections.
    # wqkvT[:, 128*i:128*(i+1)] = (g*W_i)^T ; aug3 row0 = -(W_i g) (multiplies
    # mean*rstd), row1 = W_i @ b.
    wg = [Wq * g[None, :], Wk * g[None, :], Wv * g[None, :]]
    wqkvT = np.ascontiguousarray(np.concatenate([w.T for w in wg], axis=1)).astype(np.float16)
    aug3 = np.ascontiguousarray(np.stack([
        np.concatenate([-w.sum(axis=1) for w in wg]),
        np.concatenate([W @ b for W in (Wq, Wk, Wv)]),
    ])).astype(np.float16)
    woT = np.ascontiguousarray(Wo.T).astype(np.float16)

    return [
        {'x': x[i], 'wqkvT': wqkvT, 'aug3': aug3, 'woT': woT, 'bo': bo}
        for i in range(B)
    ]


def kernel(**inputs):
    from concourse.bass_utils import run_bass_kernel_spmd

    nc = _get_nc()
    res = run_bass_kernel_spmd(nc, _prep_in_maps(inputs), list(range(B)))
    return np.stack([res.results[i]['y'] for i in range(B)]).astype(np.float32)


# revision 30
# speedup vs baseline: 1.1853x; 1.1853x over previous
"""Trainium2 Bass kernel for CrossAttention (LayerNorm + self-attention + 1x1 conv + residual).

Sharding: data-parallel over batch - B=8, one batch element per NeuronCore.

Per-core design:
 - ScalarE exp stream is the roofline (H*L^2 = 16.8M exps @ 1 elem/cycle/lane
   @1.2GHz ~= 110us + per-instruction overhead); everything else is pipelined
   underneath it, so the optimization targets are the head (time before the
   first exp) and the tail.
 - weights are folded on the host: (g*W)^T fp16, LN-mean/bias augmentation rows,
   Wo^T fp16 - the device does no weight transposes or scaling at all.
 - LayerNorm + QKV projection run in 4 column-chunks of 512 software-pipelined
   INTO the attention loop: chunk 0 gates the first exp (~15us head), chunks
   1-3 are emitted between the first exp slots of d=0 and execute underneath
   the exp stream on PE/DVE/Pool.
 - all matmuls fp16 (1 cycle/row on PE), fp32 PSUM accumulation; softmax
   without max-subtraction (logits are O(1)).
 - scores for 2 heads x 512 queries per [128,1024] PSUM tile, 3-deep rotation;
   Z/AV matmuls run one pipeline position behind the exps.
 - a short chained-matmul warmup burst runs during the x-DMA wait so the PE
   HAM clock-gate opens (1.2 -> 2.4 GHz) before the attention stream starts.
 - single ACT table load: the ln/exp loads are steered to the combined
   natural_log_exp_and_others set (see _patch_act_tables).
"""
import numpy as np

B, C, L = 8, 256, 2048
H, DH = 4, 32
HID = H * DH           # 128
EPS = 1e-5
SCALE = DH ** -0.5
P = 128                # partitions
DQ = 512               # query tile (free dim of score matmuls)
ND = L // DQ           # 4 d-tiles
NE = L // P            # 16 key tiles of 128
CW = 512               # LN/proj column chunk width
NC = L // CW           # 4 chunks

_cached = None


def _patch_act_tables():
    """Steer the greedy ACT-table-load inserter to the combined ln+exp set.

    The inserter picks the first act_func_set containing the needed function;
    'exp' hits exp_and_others and 'ln' hits natural_log, so an interleaved
    ln/exp sequence reloads tables on every switch (1.28us each). Dropping
    those functions from the other sets (indices preserved) makes both
    resolve to natural_log_exp_and_others -> one load for the whole kernel.
    Set ids are positional, so only contents may be edited.
    """
    import concourse.bacc as bacc

    if getattr(bacc, '_act_tables_patched', False):
        return
    orig = bacc.get_activation_tables

    def patched(arch):
        tables = {k: set(v) for k, v in orig(arch).items()}
        if 'natural_log_exp_and_others' in tables:
            combined = tables['natural_log_exp_and_others']
            for name, fns in tables.items():
                if name != 'natural_log_exp_and_others':
                    fns -= {f for f in fns if f in combined
                            and str(f).lower().endswith(('exp', 'ln', 'square'))}
        return tables

    bacc.get_activation_tables = patched
    bacc._act_tables_patched = True


def _build():
    import concourse.bass as bass
    import concourse.bacc as bacc
    import concourse.tile as tile
    from concourse import mybir

    _patch_act_tables()

    f32 = mybir.dt.float32
    f16 = mybir.dt.float16
    AF = mybir.ActivationFunctionType
    OP = mybir.AluOpType

    nc = bacc.Bacc('TRN2', target_bir_lowering=False, debug=False, num_devices=B)

    xd = nc.dram_tensor('x', [C, L], f32, kind='ExternalInput').ap()
    wqkvd = nc.dram_tensor('wqkvT', [C, 3 * HID], f16, kind='ExternalInput').ap()
    aug3d = nc.dram_tensor('aug3', [2, 3 * HID], f16, kind='ExternalInput').ap()
    wod = nc.dram_tensor('woT', [HID, C], f16, kind='ExternalInput').ap()
    bod = nc.dram_tensor('bo', [C, 1], f32, kind='ExternalInput').ap()
    yd = nc.dram_tensor('y', [C, L], f32, kind='ExternalOutput').ap()

    with tile.TileContext(nc) as tc:
        with (
            tc.tile_pool(name='const', bufs=1) as const,
            tc.tile_pool(name='big', bufs=1) as big,
            tc.tile_pool(name='sc', bufs=2) as sc,
            tc.tile_pool(name='apool', bufs=6) as apool,
            tc.tile_pool(name='tpool', bufs=2) as tpool,
            tc.tile_pool(name='psA', bufs=3, space='PSUM') as psA,
            tc.tile_pool(name='psB', bufs=2, space='PSUM') as psB,
        ):
            # ---- persistent tiles ----
            x0 = big.tile([P, L], f32, tag='x0')
            x1 = big.tile([P, L], f32, tag='x1')
            xs0 = big.tile([P, L], f16, tag='xs0')
            xs1 = big.tile([P, L], f16, tag='xs1')
            qt = big.tile([HID, L], f16, tag='qt')
            kt = big.tile([HID, L], f16, tag='kt')
            vsb = big.tile([P, NE, HID], f16, tag='vsb')
            aug2 = big.tile([2, L], f16, tag='aug2')

            wqkv0 = const.tile([P, 3 * HID], f16, tag='wqkv0')
            wqkv1 = const.tile([P, 3 * HID], f16, tag='wqkv1')
            aug3sb = const.tile([2, 3 * HID], f16, tag='aug3sb')
            woT = const.tile([HID, C], f16, tag='woT')
            boc = [const.tile([P, 1], f32, tag=f'bo{c}', name=f'bo{c}') for c in range(2)]
            ones16 = const.tile([P, P], f16, tag='ones16')
            epst = const.tile([P, 1], f32, tag='epst')
            warm_src = const.tile([P, 512], f16, tag='wsrc')

            WIX = {'q': 0, 'k': 1, 'v': 2}

            def wsl(name, c):
                i = WIX[name]
                t = wqkv0 if c == 0 else wqkv1
                return t[:, i * HID:(i + 1) * HID]

            def asl(name):
                i = WIX[name]
                return aug3sb[:, i * HID:(i + 1) * HID]

            # ---- queue heads: x chunk0 first on SyncE; weights on Tensor/Vector
            # queues; ScalarE queue stays empty so the ACT table load fires
            # immediately and the LN chain is never behind DMA issues ----
            nc.sync.dma_start(out=x0[:, 0:CW], in_=xd[0:P, 0:CW])
            nc.sync.dma_start(out=x1[:, 0:CW], in_=xd[P:C, 0:CW])
            nc.sync.dma_start(out=x0[:, CW:L], in_=xd[0:P, CW:L])
            nc.sync.dma_start(out=x1[:, CW:L], in_=xd[P:C, CW:L])
            nc.scalar.dma_start(out=wqkv0, in_=wqkvd[0:P, :])
            nc.scalar.dma_start(out=wqkv1, in_=wqkvd[P:C, :])
            nc.vector.memset(epst, EPS)
            nc.scalar.dma_start(out=woT, in_=wod)
            nc.scalar.dma_start(out=aug3sb, in_=aug3d)
            nc.scalar.dma_start(out=boc[0], in_=bod[0:P, :])
            nc.scalar.dma_start(out=boc[1], in_=bod[P:C, :])
            nc.gpsimd.memset(warm_src, 0.5)
            nc.gpsimd.memset(ones16, 1.0)
            nc.gpsimd.memset(aug2, 1.0)  # row 1 stays 1.0; row 0 overwritten per chunk

            # ---- PE warmup: chained dead matmuls during the x-DMA wait; the
            # HAM clock gate needs >=3.4us of sustained busy to open
            # (1.2 -> 2.4 GHz), and a ~3.4us idle re-throttles it, so the
            # initial burst is >4us and short filler bursts are interleaved
            # with the sparse chunk-0 PE work to bridge until the attention
            # stream (which then keeps PE dense) ----
            # The warm matmuls write garbage into d=0's zp accumulator
            # (pre-allocated here): zp0 is read by the softmax reciprocal so
            # the chain is output-rooted (a dedicated scratch tile gets
            # dead-code-eliminated, silently dropping the warmup), and the
            # first real ZAV matmul has start=True, which clears the bank.
            zp0 = psB.tile([P, DQ], f32, tag='acc', name='zp0')
            op0 = psB.tile([P, DQ], f32, tag='acc', name='op0')
            NWARM = 10
            for i in range(NWARM):
                nc.tensor.matmul(zp0, lhsT=ones16, rhs=warm_src,
                                 start=(i == 0), stop=False)

            def emit_warm_filler(n, close=False):
                for i in range(n):
                    nc.tensor.matmul(zp0, lhsT=ones16, rhs=warm_src,
                                     start=False, stop=(close and i == n - 1))

            # ---- per-chunk LN + projections ----
            def emit_chunk_A(c, act_sq=False):
                """stats -> mean, rstd for columns [c*CW, (c+1)*CW).

                act_sq: compute the squares on the (idle) ScalarE instead of
                Pool - used for chunk 0 only, where the squares sit on the
                critical path to the first exp and ACT has nothing to do yet;
                Square lives in the combined ln/exp table set (see
                _patch_act_tables) so it costs no extra table load.
                """
                sl = slice(c * CW, (c + 1) * CW)
                xb0 = sc.tile([P, CW], f16, tag='xb', name=f'xb0_{c}')
                xb1 = sc.tile([P, CW], f16, tag='xb2', name=f'xb1_{c}')
                xq0 = sc.tile([P, CW], f16, tag='xq', name=f'xq0_{c}')
                xq1 = sc.tile([P, CW], f16, tag='xq2', name=f'xq1_{c}')
                if act_sq:
                    # chunk 0: casts on the (empty) DVE queue, squares on the
                    # idle ScalarE
                    nc.vector.tensor_copy(xb0, x0[:, sl])
                    nc.vector.tensor_copy(xb1, x1[:, sl])
                    nc.scalar.activation(xq0, x0[:, sl], AF.Square)
                    nc.scalar.activation(xq1, x1[:, sl], AF.Square)
                else:
                    # chunks 1-3: xb casts on ScalarE, NOT DVE. They are the
                    # only chunk ops ready early (input: just x), so the Tile
                    # scheduler otherwise promotes them ahead of chunk 0's
                    # veps in the in-order DVE queue, delaying the first Ln
                    # and first exp by ~4us. On ACT they slot into the d=0
                    # stall windows (ACT idles there waiting on the lagging
                    # PE), so they are effectively free. (Pool hosting was
                    # tried twice and serializes the chunk pipeline badly.)
                    nc.scalar.copy(xb0, x0[:, sl])
                    nc.scalar.copy(xb1, x1[:, sl])
                    nc.gpsimd.tensor_mul(xq0, x0[:, sl], x0[:, sl])
                    nc.gpsimd.tensor_mul(xq1, x1[:, sl], x1[:, sl])
                st = psA.tile([P, 1024], f32, tag='psA', name=f'st{c}')
                nc.tensor.matmul(st[:, 0:CW], lhsT=ones16, rhs=xb0, start=True, stop=False)
                nc.tensor.matmul(st[:, 0:CW], lhsT=ones16, rhs=xb1, start=False, stop=True)
                nc.tensor.matmul(st[:, CW:], lhsT=ones16, rhs=xq0, start=True, stop=False)
                nc.tensor.matmul(st[:, CW:], lhsT=ones16, rhs=xq1, start=False, stop=True)
                mean = sc.tile([P, CW], f32, tag='mean', name=f'mean{c}')
                nc.vector.tensor_scalar_mul(mean, st[:, 0:CW], 1.0 / C)
                msq = sc.tile([P, CW], f32, tag='msq', name=f'msq{c}')
                nc.gpsimd.tensor_mul(msq, mean, mean)
                veps = sc.tile([P, CW], f32, tag='veps', name=f'veps{c}')
                nc.vector.scalar_tensor_tensor(veps, in0=st[:, CW:], scalar=1.0 / C,
                                               in1=msq, op0=OP.mult, op1=OP.subtract)
                lv = sc.tile([P, CW], f32, tag='lv', name=f'lv{c}')
                nc.scalar.activation(lv, veps, AF.Ln, bias=epst)
                # rstd = exp(-0.5*ln(var+eps)); single combined ln/exp ACT table
                rstd = sc.tile([P, CW], f32, tag='rstd', name=f'rstd{c}')
                nc.scalar.activation(rstd, lv, AF.Exp, scale=-0.5)
                return mean, rstd

            def emit_proj(name, dst, c):
                sl = slice(c * CW, (c + 1) * CW)
                pt = psA.tile([P, CW], f32, tag='psA', name=f'pt_{name}{c}')
                nc.tensor.matmul(pt, lhsT=wsl(name, 0), rhs=xs0[:, sl], start=True, stop=False)
                nc.tensor.matmul(pt, lhsT=wsl(name, 1), rhs=xs1[:, sl], start=False, stop=False)
                nc.tensor.matmul(pt, lhsT=asl(name), rhs=aug2[:, sl], start=False, stop=True)
                nc.vector.tensor_copy(dst[:, sl], pt)

            def emit_chunk_B1(c, mean, rstd):
                """xs, aug row, q projection for the chunk."""
                sl = slice(c * CW, (c + 1) * CW)
                nc.vector.tensor_mul(xs0[:, sl], x0[:, sl], rstd)
                nc.vector.tensor_mul(xs1[:, sl], x1[:, sl], rstd)
                nc.gpsimd.tensor_mul(aug2[0:1, sl], mean[0:1, :], rstd[0:1, :])
                emit_proj('q', qt, c)

            def emit_chunk_B2(c):
                """k projection for the chunk."""
                emit_proj('k', kt, c)

            def emit_chunk_B(c, mean, rstd):
                emit_chunk_B1(c, mean, rstd)
                emit_chunk_B2(c)

            def emit_v_chunk(e):
                se = slice(e * P, (e + 1) * P)
                vp = psA.tile([P, HID], f32, tag='psA', name=f'vp{e}')
                nc.tensor.matmul(vp, lhsT=xs0[:, se], rhs=wsl('v', 0), start=True, stop=False)
                nc.tensor.matmul(vp, lhsT=xs1[:, se], rhs=wsl('v', 1), start=False, stop=False)
                nc.tensor.matmul(vp, lhsT=aug2[:, se], rhs=asl('v'), start=False, stop=True)
                nc.vector.tensor_copy(vsb[:, e, :], vp)

            # ---- attention: pipelined over (d, e); ACT exp stream is the roofline ----
            def emit_qk_exp(d, e):
                sd = slice(d * DQ, (d + 1) * DQ)
                se = slice(e * P, (e + 1) * P)
                ats = []
                for half in range(2):
                    sp = psA.tile([P, 1024], f32, tag='psA', name=f'sp{d}_{e}_{half}')
                    for hh in range(2):
                        h = half * 2 + hh
                        hp = slice(32 * h, 32 * h + 32)
                        nc.tensor.matmul(sp[:, hh * DQ:(hh + 1) * DQ], lhsT=kt[hp, se],
                                         rhs=qt[hp, sd], start=True, stop=True,
                                         tile_position=(32 * h, 0))
                    at = apool.tile([P, 1024], f16, tag='at', name=f'at{d}_{e}_{half}')
                    nc.scalar.activation(at, sp, AF.Exp, scale=SCALE)
                    ats.append(at)
                return ats

            def emit_zav(d, e, ats, zp, op_):
                for half in range(2):
                    at = ats[half]
                    for hh in range(2):
                        h = half * 2 + hh
                        hp = slice(32 * h, 32 * h + 32)
                        asl_ = slice(hh * DQ, (hh + 1) * DQ)
                        nc.tensor.matmul(zp[hp, :], lhsT=ones16[:, 0:32], rhs=at[:, asl_],
                                         start=(e == 0), stop=(e == NE - 1),
                                         tile_position=(0, 32 * h))
                        nc.tensor.matmul(op_[hp, :], lhsT=vsb[:, e, hp], rhs=at[:, asl_],
                                         start=(e == 0), stop=(e == NE - 1),
                                         tile_position=(0, 32 * h))

            def emit_norm_tail(d, zp, op_):
                # ~18-bit reciprocal, 5x faster than nc.vector.reciprocal --
                # plenty for softmax denominators (Z is O(100..3000), positive)
                rz = tpool.tile([P, DQ], f32, tag='rz', name=f'rz{d}')
                nc.vector.reciprocal_approx_fast(out=rz, in_=zp)
                onorm = tpool.tile([P, DQ], f16, tag='onorm', name=f'onorm{d}')
                nc.vector.tensor_mul(onorm, op_, rz)
                return onorm

            def emit_proj_tail(d, onorm):
                sd = slice(d * DQ, (d + 1) * DQ)
                yp = psA.tile([P, 1024], f32, tag='psA', name=f'yp{d}')
                for c in range(2):
                    nc.tensor.matmul(yp[:, c * DQ:(c + 1) * DQ], lhsT=woT[:, c * P:(c + 1) * P],
                                     rhs=onorm, start=True, stop=True)
                for c in range(2):
                    ysb = tpool.tile([P, DQ], f32, tag='ysb', name=f'ysb{d}_{c}')
                    xc = x0 if c == 0 else x1
                    nc.vector.scalar_tensor_tensor(ysb, in0=yp[:, c * DQ:(c + 1) * DQ],
                                                   scalar=boc[c], in1=xc[:, sd],
                                                   op0=OP.add, op1=OP.add)
                    nc.sync.dma_start(out=yd[c * P:(c + 1) * P, sd], in_=ysb)

            # ---- chunk 0 gates the head; chunks 1-3 pipeline into d=0.
            # Warm fillers bridge the PE-idle stretches of the chunk-0 chain
            # (stats -> LN -> xs -> proj) so the HAM gate, opened by the
            # initial burst, is still open when the attention stream starts. ----
            m0, r0 = emit_chunk_A(0, act_sq=True)
            emit_warm_filler(6)
            emit_chunk_B(0, m0, r0)
            emit_warm_filler(3)
            for e in range(4):
                emit_v_chunk(e)
            emit_warm_filler(3, close=True)

            pend = None          # (c, mean, rstd) between chunk A and B
            prev = None          # (d, e, ats) whose ZAV is emitted one position late
            zp = op_ = None      # PSUM accumulators of prev's d
            pending_norm = None  # (d, onorm) waiting for its out-projection
            for d in range(ND):
                for e in range(NE):
                    # LN chunk pipeline for chunks 1-3, spread over 3 slots of
                    # d=0 so a (possibly still cold) PE is never oversubscribed
                    # in any one slot: A at e=4c-3, B split q/k at e=4c-2/4c-1.
                    # Emitted before this slot's QK/exp so the chunk's Ln/rstd
                    # sit in the ACT queue ahead of the exps that depend on
                    # the chunk's kt (needed first at e=4c).
                    if d == 0 and e in (1, 5, 9):
                        c = (e + 3) // 4
                        pend = (c,) + emit_chunk_A(c)
                    elif d == 0 and e in (2, 6, 10):
                        emit_chunk_B1(pend[0], pend[1], pend[2])
                    elif d == 0 and e in (3, 7, 11):
                        emit_chunk_B2(pend[0])
                        pend = None
                    ats = emit_qk_exp(d, e)
                    if d == 0 and e >= 4:
                        emit_v_chunk(e)
                    if prev is not None:
                        emit_zav(prev[0], prev[1], prev[2], zp, op_)
                        if prev[1] == NE - 1:
                            pending_norm = (prev[0], emit_norm_tail(prev[0], zp, op_))
                    if e == 0:
                        if d == 0:
                            zp, op_ = zp0, op0  # pre-allocated (warm-chain target)
                        else:
                            # allocate this d's accumulators AFTER the previous
                            # d's reciprocal was emitted (correct WAR on psB)
                            zp = psB.tile([P, DQ], f32, tag='acc', name=f'zp{d}')
                            op_ = psB.tile([P, DQ], f32, tag='acc', name=f'op{d}')
                    elif e == 2 and pending_norm is not None:
                        emit_proj_tail(pending_norm[0], pending_norm[1])
                        pending_norm = None
                    prev = (d, e, ats)
            # drain
            emit_zav(prev[0], prev[1], prev[2], zp, op_)
            onorm_last = emit_norm_tail(ND - 1, zp, op_)
            emit_proj_tail(ND - 1, onorm_last)

    nc.compile()
    return nc


def _get_nc():
    global _cached
    if _cached is None:
        _cached = _build()
    return _cached


def _prep_in_maps(inputs):
    x = np.ascontiguousarray(np.asarray(inputs['x'], dtype=np.float32))
    g = np.asarray(inputs['g'], dtype=np.float32).reshape(C)
    b = np.asarray(inputs['b'], dtype=np.float32).reshape(C)
    Wq = np.asarray(inputs['Wq'], dtype=np.float32)
    Wk = np.asarray(inputs['Wk'], dtype=np.float32)
    Wv = np.asarray(inputs['Wv'], dtype=np.float32)
    Wo = np.asarray(inputs['Wo'], dtype=np.float32)
    bo = np.ascontiguousarray(np.asarray(inputs['bo'], dtype=np.float32).reshape(C, 1))

    # host-side weight folding: LN affine (g, b) into the QKV projections.
    # wqkvT[:, 128*i:128*(i+1)] = (g*W_i)^T ; aug3 row0 = -(W_i g) (multiplies
    # mean*rstd), row1 = W_i @ b.
    wg = [Wq * g[None, :], Wk * g[None, :], Wv * g[None, :]]
    wqkvT = np.ascontiguousarray(np.concatenate([w.T for w in wg], axis=1)).astype(np.float16)
    aug3 = np.ascontiguousarray(np.stack([
        np.concatenate([-w.sum(axis=1) for w in wg]),
        np.concatenate([W @ b for W in (Wq, Wk, Wv)]),
    ])).astype(np.float16)
    woT = np.ascontiguousarray(Wo.T).astype(np.float16)

    return [
        {'x': x[i], 'wqkvT': wqkvT, 'aug3': aug3, 'woT': woT, 'bo': bo}
        for i in range(B)
    ]


def kernel(**inputs):
    from concourse.bass_utils import run_bass_kernel_spmd

    nc = _get_nc()
    res = run_bass_kernel_spmd(nc, _prep_in_maps(inputs), list(range(B)))
    return np.stack([res.results[i]['y'] for i in range(B)]).astype(np.float32)


# revision 31
# speedup vs baseline: 1.1876x; 1.0019x over previous
"""Trainium2 Bass kernel for CrossAttention (LayerNorm + self-attention + 1x1 conv + residual).

Sharding: data-parallel over batch - B=8, one batch element per NeuronCore.

Per-core design:
 - ScalarE exp stream is the roofline (H*L^2 = 16.8M exps @ 1 elem/cycle/lane
   @1.2GHz ~= 110us + per-instruction overhead); everything else is pipelined
   underneath it, so the optimization targets are the head (time before the
   first exp) and the tail.
 - weights are folded on the host: (g*W)^T fp16, LN-mean/bias augmentation rows,
   Wo^T fp16 - the device does no weight transposes or scaling at all.
 - LayerNorm + QKV projection run in 4 column-chunks of 512 software-pipelined
   INTO the attention loop: chunk 0 gates the first exp (~15us head), chunks
   1-3 are emitted between the first exp slots of d=0 and execute underneath
   the exp stream on PE/DVE/Pool.
 - all matmuls fp16 (1 cycle/row on PE), fp32 PSUM accumulation; softmax
   without max-subtraction (logits are O(1)).
 - scores for 2 heads x 512 queries per [128,1024] PSUM tile, 3-deep rotation;
   Z/AV matmuls run one pipeline position behind the exps.
 - a short chained-matmul warmup burst runs during the x-DMA wait so the PE
   HAM clock-gate opens (1.2 -> 2.4 GHz) before the attention stream starts.
 - single ACT table load: the ln/exp loads are steered to the combined
   natural_log_exp_and_others set (see _patch_act_tables).
"""
import numpy as np

B, C, L = 8, 256, 2048
H, DH = 4, 32
HID = H * DH           # 128
EPS = 1e-5
SCALE = DH ** -0.5
P = 128                # partitions
DQ = 512               # query tile (free dim of score matmuls)
ND = L // DQ           # 4 d-tiles
NE = L // P            # 16 key tiles of 128
CW = 512               # LN/proj column chunk width
NC = L // CW           # 4 chunks

_cached = None


def _patch_act_tables():
    """Steer the greedy ACT-table-load inserter to the combined ln+exp set.

    The inserter picks the first act_func_set containing the needed function;
    'exp' hits exp_and_others and 'ln' hits natural_log, so an interleaved
    ln/exp sequence reloads tables on every switch (1.28us each). Dropping
    those functions from the other sets (indices preserved) makes both
    resolve to natural_log_exp_and_others -> one load for the whole kernel.
    Set ids are positional, so only contents may be edited.
    """
    import concourse.bacc as bacc

    if getattr(bacc, '_act_tables_patched', False):
        return
    orig = bacc.get_activation_tables

    def patched(arch):
        tables = {k: set(v) for k, v in orig(arch).items()}
        if 'natural_log_exp_and_others' in tables:
            combined = tables['natural_log_exp_and_others']
            for name, fns in tables.items():
                if name != 'natural_log_exp_and_others':
                    fns -= {f for f in fns if f in combined
                            and str(f).lower().endswith(('exp', 'ln', 'square'))}
        return tables

    bacc.get_activation_tables = patched
    bacc._act_tables_patched = True


def _build():
    import concourse.bass as bass
    import concourse.bacc as bacc
    import concourse.tile as tile
    from concourse import mybir

    _patch_act_tables()

    f32 = mybir.dt.float32
    f16 = mybir.dt.float16
    AF = mybir.ActivationFunctionType
    OP = mybir.AluOpType

    nc = bacc.Bacc('TRN2', target_bir_lowering=False, debug=False, num_devices=B)

    xd = nc.dram_tensor('x', [C, L], f32, kind='ExternalInput').ap()
    wqkvd = nc.dram_tensor('wqkvT', [C, 3 * HID], f16, kind='ExternalInput').ap()
    aug3d = nc.dram_tensor('aug3', [2, 3 * HID], f16, kind='ExternalInput').ap()
    wod = nc.dram_tensor('woT', [HID, C], f16, kind='ExternalInput').ap()
    bod = nc.dram_tensor('bo', [C, 1], f32, kind='ExternalInput').ap()
    yd = nc.dram_tensor('y', [C, L], f32, kind='ExternalOutput').ap()

    with tile.TileContext(nc) as tc:
        with (
            tc.tile_pool(name='const', bufs=1) as const,
            tc.tile_pool(name='big', bufs=1) as big,
            tc.tile_pool(name='sc', bufs=2) as sc,
            tc.tile_pool(name='apool', bufs=6) as apool,
            tc.tile_pool(name='tpool', bufs=2) as tpool,
            tc.tile_pool(name='psA', bufs=3, space='PSUM') as psA,
            tc.tile_pool(name='psB', bufs=2, space='PSUM') as psB,
        ):
            # ---- persistent tiles ----
            x0 = big.tile([P, L], f32, tag='x0')
            x1 = big.tile([P, L], f32, tag='x1')
            xs0 = big.tile([P, L], f16, tag='xs0')
            xs1 = big.tile([P, L], f16, tag='xs1')
            qt = big.tile([HID, L], f16, tag='qt')
            kt = big.tile([HID, L], f16, tag='kt')
            vsb = big.tile([P, NE, HID], f16, tag='vsb')
            aug2 = big.tile([2, L], f16, tag='aug2')

            wqkv0 = const.tile([P, 3 * HID], f16, tag='wqkv0')
            wqkv1 = const.tile([P, 3 * HID], f16, tag='wqkv1')
            aug3sb = const.tile([2, 3 * HID], f16, tag='aug3sb')
            woT = const.tile([HID, C], f16, tag='woT')
            boc = [const.tile([P, 1], f32, tag=f'bo{c}', name=f'bo{c}') for c in range(2)]
            ones16 = const.tile([P, P], f16, tag='ones16')
            epst = const.tile([P, 1], f32, tag='epst')
            warm_src = const.tile([P, 512], f16, tag='wsrc')

            WIX = {'q': 0, 'k': 1, 'v': 2}

            def wsl(name, c):
                i = WIX[name]
                t = wqkv0 if c == 0 else wqkv1
                return t[:, i * HID:(i + 1) * HID]

            def asl(name):
                i = WIX[name]
                return aug3sb[:, i * HID:(i + 1) * HID]

            # ---- queue heads: x chunk0 first on SyncE; weights on Tensor/Vector
            # queues; ScalarE queue stays empty so the ACT table load fires
            # immediately and the LN chain is never behind DMA issues ----
            nc.sync.dma_start(out=x0[:, 0:CW], in_=xd[0:P, 0:CW])
            nc.sync.dma_start(out=x1[:, 0:CW], in_=xd[P:C, 0:CW])
            nc.sync.dma_start(out=x0[:, CW:L], in_=xd[0:P, CW:L])
            nc.sync.dma_start(out=x1[:, CW:L], in_=xd[P:C, CW:L])
            nc.scalar.dma_start(out=wqkv0, in_=wqkvd[0:P, :])
            nc.scalar.dma_start(out=wqkv1, in_=wqkvd[P:C, :])
            nc.vector.memset(epst, EPS)
            nc.scalar.dma_start(out=woT, in_=wod)
            nc.scalar.dma_start(out=aug3sb, in_=aug3d)
            nc.scalar.dma_start(out=boc[0], in_=bod[0:P, :])
            nc.scalar.dma_start(out=boc[1], in_=bod[P:C, :])
            nc.gpsimd.memset(warm_src, 0.5)
            nc.gpsimd.memset(ones16, 1.0)
            nc.gpsimd.memset(aug2, 1.0)  # row 1 stays 1.0; row 0 overwritten per chunk

            # ---- PE warmup: chained dead matmuls during the x-DMA wait; the
            # HAM clock gate needs >=3.4us of sustained busy to open
            # (1.2 -> 2.4 GHz), and a ~3.4us idle re-throttles it, so the
            # initial burst is >4us and short filler bursts are interleaved
            # with the sparse chunk-0 PE work to bridge until the attention
            # stream (which then keeps PE dense) ----
            # The warm matmuls write garbage into d=0's zp accumulator
            # (pre-allocated here): zp0 is read by the softmax reciprocal so
            # the chain is output-rooted (a dedicated scratch tile gets
            # dead-code-eliminated, silently dropping the warmup), and the
            # first real ZAV matmul has start=True, which clears the bank.
            zp0 = psB.tile([P, DQ], f32, tag='acc', name='zp0')
            op0 = psB.tile([P, DQ], f32, tag='acc', name='op0')
            NWARM = 10
            for i in range(NWARM):
                nc.tensor.matmul(zp0, lhsT=ones16, rhs=warm_src,
                                 start=(i == 0), stop=False)

            def emit_warm_filler(n, close=False):
                for i in range(n):
                    nc.tensor.matmul(zp0, lhsT=ones16, rhs=warm_src,
                                     start=False, stop=(close and i == n - 1))

            # ---- per-chunk LN + projections ----
            def emit_chunk_A(c, act_sq=False):
                """stats -> mean, rstd for columns [c*CW, (c+1)*CW).

                act_sq: compute the squares on the (idle) ScalarE instead of
                Pool - used for chunk 0 only, where the squares sit on the
                critical path to the first exp and ACT has nothing to do yet;
                Square lives in the combined ln/exp table set (see
                _patch_act_tables) so it costs no extra table load.
                """
                sl = slice(c * CW, (c + 1) * CW)
                xb0 = sc.tile([P, CW], f16, tag='xb', name=f'xb0_{c}')
                nc.vector.tensor_copy(xb0, x0[:, sl])
                xb1 = sc.tile([P, CW], f16, tag='xb2', name=f'xb1_{c}')
                nc.vector.tensor_copy(xb1, x1[:, sl])
                xq0 = sc.tile([P, CW], f16, tag='xq', name=f'xq0_{c}')
                xq1 = sc.tile([P, CW], f16, tag='xq2', name=f'xq1_{c}')
                if act_sq:
                    nc.scalar.activation(xq0, x0[:, sl], AF.Square)
                    nc.scalar.activation(xq1, x1[:, sl], AF.Square)
                else:
                    nc.gpsimd.tensor_mul(xq0, x0[:, sl], x0[:, sl])
                    nc.gpsimd.tensor_mul(xq1, x1[:, sl], x1[:, sl])
                st = psA.tile([P, 1024], f32, tag='psA', name=f'st{c}')
                nc.tensor.matmul(st[:, 0:CW], lhsT=ones16, rhs=xb0, start=True, stop=False)
                nc.tensor.matmul(st[:, 0:CW], lhsT=ones16, rhs=xb1, start=False, stop=True)
                nc.tensor.matmul(st[:, CW:], lhsT=ones16, rhs=xq0, start=True, stop=False)
                nc.tensor.matmul(st[:, CW:], lhsT=ones16, rhs=xq1, start=False, stop=True)
                mean = sc.tile([P, CW], f32, tag='mean', name=f'mean{c}')
                nc.vector.tensor_scalar_mul(mean, st[:, 0:CW], 1.0 / C)
                msq = sc.tile([P, CW], f32, tag='msq', name=f'msq{c}')
                nc.gpsimd.tensor_mul(msq, mean, mean)
                veps = sc.tile([P, CW], f32, tag='veps', name=f'veps{c}')
                nc.vector.scalar_tensor_tensor(veps, in0=st[:, CW:], scalar=1.0 / C,
                                               in1=msq, op0=OP.mult, op1=OP.subtract)
                lv = sc.tile([P, CW], f32, tag='lv', name=f'lv{c}')
                nc.scalar.activation(lv, veps, AF.Ln, bias=epst)
                # rstd = exp(-0.5*ln(var+eps)); single combined ln/exp ACT table
                rstd = sc.tile([P, CW], f32, tag='rstd', name=f'rstd{c}')
                nc.scalar.activation(rstd, lv, AF.Exp, scale=-0.5)
                return mean, rstd

            def emit_proj(name, dst, c):
                sl = slice(c * CW, (c + 1) * CW)
                pt = psA.tile([P, CW], f32, tag='psA', name=f'pt_{name}{c}')
                nc.tensor.matmul(pt, lhsT=wsl(name, 0), rhs=xs0[:, sl], start=True, stop=False)
                nc.tensor.matmul(pt, lhsT=wsl(name, 1), rhs=xs1[:, sl], start=False, stop=False)
                nc.tensor.matmul(pt, lhsT=asl(name), rhs=aug2[:, sl], start=False, stop=True)
                nc.vector.tensor_copy(dst[:, sl], pt)

            def emit_chunk_B1(c, mean, rstd):
                """xs, aug row, q projection for the chunk."""
                sl = slice(c * CW, (c + 1) * CW)
                nc.vector.tensor_mul(xs0[:, sl], x0[:, sl], rstd)
                nc.vector.tensor_mul(xs1[:, sl], x1[:, sl], rstd)
                nc.gpsimd.tensor_mul(aug2[0:1, sl], mean[0:1, :], rstd[0:1, :])
                emit_proj('q', qt, c)

            def emit_chunk_B2(c):
                """k projection for the chunk."""
                emit_proj('k', kt, c)

            def emit_chunk_B(c, mean, rstd):
                emit_chunk_B1(c, mean, rstd)
                emit_chunk_B2(c)

            def emit_v_chunk(e):
                se = slice(e * P, (e + 1) * P)
                vp = psA.tile([P, HID], f32, tag='psA', name=f'vp{e}')
                nc.tensor.matmul(vp, lhsT=xs0[:, se], rhs=wsl('v', 0), start=True, stop=False)
                nc.tensor.matmul(vp, lhsT=xs1[:, se], rhs=wsl('v', 1), start=False, stop=False)
                nc.tensor.matmul(vp, lhsT=aug2[:, se], rhs=asl('v'), start=False, stop=True)
                nc.vector.tensor_copy(vsb[:, e, :], vp)

            # ---- attention: pipelined over (d, e); ACT exp stream is the roofline ----
            def emit_qk_exp(d, e):
                sd = slice(d * DQ, (d + 1) * DQ)
                se = slice(e * P, (e + 1) * P)
                ats = []
                for half in range(2):
                    sp = psA.tile([P, 1024], f32, tag='psA', name=f'sp{d}_{e}_{half}')
                    for hh in range(2):
                        h = half * 2 + hh
                        hp = slice(32 * h, 32 * h + 32)
                        nc.tensor.matmul(sp[:, hh * DQ:(hh + 1) * DQ], lhsT=kt[hp, se],
                                         rhs=qt[hp, sd], start=True, stop=True,
                                         tile_position=(32 * h, 0))
                    at = apool.tile([P, 1024], f16, tag='at', name=f'at{d}_{e}_{half}')
                    nc.scalar.activation(at, sp, AF.Exp, scale=SCALE)
                    ats.append(at)
                return ats

            def emit_zav(d, e, ats, zp, op_):
                for half in range(2):
                    at = ats[half]
                    for hh in range(2):
                        h = half * 2 + hh
                        hp = slice(32 * h, 32 * h + 32)
                        asl_ = slice(hh * DQ, (hh + 1) * DQ)
                        nc.tensor.matmul(zp[hp, :], lhsT=ones16[:, 0:32], rhs=at[:, asl_],
                                         start=(e == 0), stop=(e == NE - 1),
                                         tile_position=(0, 32 * h))
                        nc.tensor.matmul(op_[hp, :], lhsT=vsb[:, e, hp], rhs=at[:, asl_],
                                         start=(e == 0), stop=(e == NE - 1),
                                         tile_position=(0, 32 * h))

            def emit_norm_tail(d, zp, op_):
                # ~18-bit reciprocal, 5x faster than nc.vector.reciprocal --
                # plenty for softmax denominators (Z is O(100..3000), positive)
                rz = tpool.tile([P, DQ], f32, tag='rz', name=f'rz{d}')
                nc.vector.reciprocal_approx_fast(out=rz, in_=zp)
                onorm = tpool.tile([P, DQ], f16, tag='onorm', name=f'onorm{d}')
                nc.vector.tensor_mul(onorm, op_, rz)
                return onorm

            def emit_proj_tail(d, onorm):
                sd = slice(d * DQ, (d + 1) * DQ)
                yp = psA.tile([P, 1024], f32, tag='psA', name=f'yp{d}')
                for c in range(2):
                    nc.tensor.matmul(yp[:, c * DQ:(c + 1) * DQ], lhsT=woT[:, c * P:(c + 1) * P],
                                     rhs=onorm, start=True, stop=True)
                for c in range(2):
                    ysb = tpool.tile([P, DQ], f32, tag='ysb', name=f'ysb{d}_{c}')
                    xc = x0 if c == 0 else x1
                    nc.vector.scalar_tensor_tensor(ysb, in0=yp[:, c * DQ:(c + 1) * DQ],
                                                   scalar=boc[c], in1=xc[:, sd],
                                                   op0=OP.add, op1=OP.add)
                    nc.sync.dma_start(out=yd[c * P:(c + 1) * P, sd], in_=ysb)

            # ---- chunk 0 gates the head; chunks 1-3 pipeline into d=0.
            # Warm fillers bridge the PE-idle stretches of the chunk-0 chain
            # (stats -> LN -> xs -> proj) so the HAM gate, opened by the
            # initial burst, is still open when the attention stream starts. ----
            m0, r0 = emit_chunk_A(0, act_sq=True)
            emit_warm_filler(6)
            emit_chunk_B(0, m0, r0)
            emit_warm_filler(3)
            for e in range(4):
                emit_v_chunk(e)
            emit_warm_filler(3, close=True)

            pend = None          # (c, mean, rstd) between chunk A and B
            prev = None          # (d, e, ats) whose ZAV is emitted one position late
            zp = op_ = None      # PSUM accumulators of prev's d
            pending_norm = None  # (d, onorm) waiting for its out-projection
            for d in range(ND):
                for e in range(NE):
                    # LN chunk pipeline for chunks 1-3, spread over 3 slots of
                    # d=0 so a (possibly still cold) PE is never oversubscribed
                    # in any one slot: A at e=4c-3, B split q/k at e=4c-2/4c-1.
                    # Emitted before this slot's QK/exp so the chunk's Ln/rstd
                    # sit in the ACT queue ahead of the exps that depend on
                    # the chunk's kt (needed first at e=4c).
                    if d == 0 and e in (1, 5, 9):
                        c = (e + 3) // 4
                        pend = (c,) + emit_chunk_A(c)
                    elif d == 0 and e in (2, 6, 10):
                        emit_chunk_B1(pend[0], pend[1], pend[2])
                    elif d == 0 and e in (3, 7, 11):
                        emit_chunk_B2(pend[0])
                        pend = None
                    ats = emit_qk_exp(d, e)
                    if d == 0 and e >= 4:
                        emit_v_chunk(e)
                    if prev is not None:
                        emit_zav(prev[0], prev[1], prev[2], zp, op_)
                        if prev[1] == NE - 1:
                            pending_norm = (prev[0], emit_norm_tail(prev[0], zp, op_))
                    if e == 0:
                        if d == 0:
                            zp, op_ = zp0, op0  # pre-allocated (warm-chain target)
                        else:
                            # allocate this d's accumulators AFTER the previous
                            # d's reciprocal was emitted (correct WAR on psB)
                            zp = psB.tile([P, DQ], f32, tag='acc', name=f'zp{d}')
                            op_ = psB.tile([P, DQ], f32, tag='acc', name=f'op{d}')
                    elif e == 2 and pending_norm is not None:
                        emit_proj_tail(pending_norm[0], pending_norm[1])
                        pending_norm = None
                    prev = (d, e, ats)
            # drain
            emit_zav(prev[0], prev[1], prev[2], zp, op_)
            onorm_last = emit_norm_tail(ND - 1, zp, op_)
            emit_proj_tail(ND - 1, onorm_last)

    nc.compile()
    return nc


def _get_nc():
    global _cached
    if _cached is None:
        _cached = _build()
    return _cached


def _prep_in_maps(inputs):
    x = np.ascontiguousarray(np.asarray(inputs['x'], dtype=np.float32))
    g = np.asarray(inputs['g'], dtype=np.float32).reshape(C)
    b = np.asarray(inputs['b'], dtype=np.float32).reshape(C)
    Wq = np.asarray(inputs['Wq'], dtype=np.float32)
    Wk = np.asarray(inputs['Wk'], dtype=np.float32)
    Wv = np.asarray(inputs['Wv'], dtype=np.float32)
    Wo = np.asarray(inputs['Wo'], dtype=np.float32)
    bo = np.ascontiguousarray(np.asarray(inputs['bo'], dtype=np.float32).reshape(C, 1))

    # host-side weight folding: LN affine (g, b) into the QKV projections.
    # wqkvT[:, 128*i:128*(i+1)] = (g*W_i)^T ; aug3 row0 = -(W_i g) (multiplies
    # mean*rstd), row1 = W_i @ b.
    wg = [Wq * g[None, :], Wk * g[None, :], Wv * g[None, :]]
    wqkvT = np.ascontiguousarray(np.concatenate([w.T for w in wg], axis=1)).astype(np.float16)
    aug3 = np.ascontiguousarray(np.stack([
        np.concatenate([-w.sum(axis=1) for w in wg]),
        np.concatenate([W @ b for W in (Wq, Wk, Wv)]),
    ])).astype(np.float16)
    woT = np.ascontiguousarray(Wo.T).astype(np.float16)

    return [
        {'x': x[i], 'wqkvT': wqkvT, 'aug3': aug3, 'woT': woT, 'bo': bo}
        for i in range(B)
    ]


def kernel(**inputs):
    from concourse.bass_utils import run_bass_kernel_spmd

    nc = _get_nc()
    res = run_bass_kernel_spmd(nc, _prep_in_maps(inputs), list(range(B)))
    return np.stack([res.results[i]['y'] for i in range(B)]).astype(np.float32)
